# revision 13
# baseline (speedup 1.0000x reference)
"""Distributed Trainium2 kernel for AdaptiveLowRank (softmax-weighted sum of
16 linear maps + LayerNorm), SPMD across 8 NeuronCores.

Strategy: data-parallel over tokens; the 16 weight matrices are sharded over
both r and output-dim so each core reads only 1/8 of Ws, locally combines
its o-slice of W_eff = sum_r softmax(rank_weights)_r * W_r (transposed on
the PE), then an AllGather gives every core the full W_eff^T for its local
x-shard matmul + LayerNorm epilogue.
"""

import numpy as np
import ml_dtypes

import concourse.bass as bass
import concourse.mybir as mybir
import concourse.tile as tile
from concourse import bacc
from concourse import bass_utils

N_CORES = 8
B, S, D = 4, 2048, 1024
R = 16
T = (B * S) // N_CORES          # tokens per core
OL = D // N_CORES               # output rows owned per core
TB = T // 128                   # token blocks per core
DB = D // 128                   # contraction blocks
LN_EPS = 1e-5
MIN_RANK, MAX_RANK = 1, 16

BF16 = mybir.dt.bfloat16
F32 = mybir.dt.float32
NP_BF16 = ml_dtypes.bfloat16

_cached = {}


def _build():
    nc = bacc.Bacc("TRN2", target_bir_lowering=False, debug=False,
                   num_devices=N_CORES)

    xs = nc.dram_tensor("xs", [T, D], BF16, kind="ExternalInput")
    ws = nc.dram_tensor("ws", [R, OL, D], BF16, kind="ExternalInput")
    probs_in = nc.dram_tensor("probs", [128, R], F32, kind="ExternalInput")
    maskc_in = nc.dram_tensor("maskc", [128, TB], F32, kind="ExternalInput")
    gamma_in = nc.dram_tensor("gammab", [128, D], BF16, kind="ExternalInput")
    beta_in = nc.dram_tensor("betab", [128, D], BF16, kind="ExternalInput")
    ident_in = nc.dram_tensor("ident", [128, 128], BF16, kind="ExternalInput")
    out = nc.dram_tensor("out", [T, D], F32, kind="ExternalOutput")

    with tile.TileContext(nc) as tc:
        with (
            tc.tile_pool(name="dram", bufs=1, space="DRAM") as dram,
            tc.tile_pool(name="consts", bufs=1) as consts,
            tc.tile_pool(name="wld", bufs=4) as wld,
            tc.tile_pool(name="wacc", bufs=2) as wacc,
            tc.tile_pool(name="wtsb", bufs=2) as wtsb,
            tc.tile_pool(name="xt", bufs=1) as xtp,
            tc.tile_pool(name="wmov", bufs=1) as wmov,
            tc.tile_pool(name="psum_t", bufs=2, space="PSUM") as psum_t,
            tc.tile_pool(name="psum_y", bufs=4, space="PSUM") as psum_y,
            tc.tile_pool(name="ln", bufs=3) as lnp,
            tc.tile_pool(name="stats", bufs=2) as stats,
        ):
            cc_in = dram.tile([D, OL], BF16)
            cc_out = dram.tile([N_CORES * D, OL], BF16)

            probs = consts.tile([128, R], F32)
            nc.sync.dma_start(probs[:], probs_in[:])
            maskc = consts.tile([128, TB], F32)
            nc.sync.dma_start(maskc[:], maskc_in[:])
            gammab = consts.tile([128, D], BF16)
            nc.sync.dma_start(gammab[:], gamma_in[:])
            betab = consts.tile([128, D], BF16)
            nc.sync.dma_start(betab[:], beta_in[:])
            ident = consts.tile([128, 128], BF16)
            nc.sync.dma_start(ident[:], ident_in[:])

            # ---- phase A: acc[o_l, d] = sum_r p_r * W_r[o_slice, d] ----
            acc = None
            for r in range(R):
                w_r = wld.tile([OL, D], BF16)
                nc.sync.dma_start(w_r[:], ws[r])
                if acc is None:
                    acc = wacc.tile([OL, D], BF16, tag="acc")
                    nc.vector.tensor_scalar(
                        acc[:], w_r[:], probs[:, 0:1], None,
                        mybir.AluOpType.mult)
                else:
                    nxt = wacc.tile([OL, D], BF16, tag="acc")
                    nc.vector.scalar_tensor_tensor(
                        nxt[:], w_r[:], probs[:, r:r + 1], acc[:],
                        mybir.AluOpType.mult, mybir.AluOpType.add)
                    acc = nxt

            # ---- phase A2: transpose acc -> cc_in[d, o_l] (PE) ----
            for k in range(DB):
                pt = psum_t.tile([128, 128], BF16)
                nc.tensor.transpose(pt[:], acc[:, k * 128:(k + 1) * 128],
                                    ident[:])
                wt_sb = wtsb.tile([128, 128], BF16)
                nc.scalar.copy(wt_sb[:], pt[:])
                nc.sync.dma_start(cc_in[k * 128:(k + 1) * 128, :], wt_sb[:])

            # ---- phase B: AllGather W_eff^T slices ----
            nc.gpsimd.collective_compute(
                "AllGather",
                mybir.AluOpType.bypass,
                replica_groups=[list(range(N_CORES))],
                ins=[cc_in[:].opt()],
                outs=[cc_out[:].opt()],
            )

            # ---- phase D: transpose x shard (DMA xbar) ----
            xt = []
            for k in range(DB):
                t_ = xtp.tile([128, T], BF16, tag=f"xt{k}")
                nc.sync.dma_start(t_[:], xs[:, k * 128:(k + 1) * 128],
                                  transpose=True)
                xt.append(t_)

            # ---- phase C: load moving W tiles [d', (r ol)] per d-block ----
            # cc_out rows: r*D + k*128 + d', cols: ol
            cc_view = cc_out[:].rearrange("(r k p) f -> k p r f",
                                          r=N_CORES, k=DB)
            wm = []
            for k in range(DB):
                t_ = wmov.tile([128, D], BF16, tag=f"wm{k}")
                dst = t_[:].rearrange("p (r f) -> p r f", r=N_CORES)
                nc.sync.dma_start(dst, cc_view[k])
                wm.append(t_)

            # ---- phase E+F: matmuls + LayerNorm per token block ----
            inv_d = 1.0 / D
            for t in range(TB):
                ph0 = psum_y.tile([128, 512], F32, tag="py")
                ph1 = psum_y.tile([128, 512], F32, tag="py")
                ph = [ph0, ph1]
                for k in range(DB):
                    lhsT = xt[k][:, t * 128:(t + 1) * 128]
                    for h in range(2):
                        nc.tensor.matmul(ph[h][:], lhsT,
                                         wm[k][:, h * 512:(h + 1) * 512],
                                         start=(k == 0), stop=(k == DB - 1))

                # stats: sums via ACT copy(+accum), sumsq via ACT square(+accum)
                st = stats.tile([128, 12], F32, tag="st")
                junk = lnp.tile([128, D], BF16, tag="junk")
                for h in range(2):
                    nc.scalar.activation(
                        junk[:, h * 512:(h + 1) * 512], ph[h][:],
                        mybir.ActivationFunctionType.Square,
                        accum_out=st[:, 2 + h:3 + h])
                    nc.scalar.activation(
                        junk[:, h * 512:(h + 1) * 512], ph[h][:],
                        mybir.ActivationFunctionType.Copy,
                        accum_out=st[:, h:h + 1])
                # mean = (s0+s1)/D
                nc.vector.tensor_scalar(
                    st[:, 4:5], st[:, 0:1], st[:, 1:2], inv_d,
                    mybir.AluOpType.add, mybir.AluOpType.mult)
                # var = (q0+q1)/D - mean^2
                nc.vector.tensor_scalar(
                    st[:, 8:9], st[:, 4:5], st[:, 4:5], -1.0,
                    mybir.AluOpType.mult, mybir.AluOpType.mult)
                nc.vector.tensor_scalar(
                    st[:, 9:10], st[:, 2:3], st[:, 3:4], inv_d,
                    mybir.AluOpType.add, mybir.AluOpType.mult)
                nc.vector.tensor_scalar(
                    st[:, 5:6], st[:, 9:10], st[:, 8:9], None,
                    mybir.AluOpType.add)
                # a_pre = m^2*var + eps ; rs = rsqrt(a_pre); a = m*rs
                m = maskc[:, t:t + 1]
                msq = stats.tile([128, 1], F32, tag="msq")
                nc.vector.tensor_scalar(
                    msq[:], m, m, None, mybir.AluOpType.mult)
                nc.vector.tensor_scalar(
                    st[:, 6:7], st[:, 5:6], msq[:], LN_EPS,
                    mybir.AluOpType.mult, mybir.AluOpType.add)
                sq = stats.tile([128, 1], F32, tag="sq")
                nc.scalar.sqrt(sq[:], st[:, 6:7])
                nc.vector.reciprocal(st[:, 7:8], sq[:])
                ab = stats.tile([128, 2], F32, tag="ab")
                nc.vector.tensor_scalar(
                    ab[:, 0:1], st[:, 7:8], m, None, mybir.AluOpType.mult)
                nc.vector.tensor_scalar(
                    ab[:, 1:2], ab[:, 0:1], st[:, 4:5], -1.0,
                    mybir.AluOpType.mult, mybir.AluOpType.mult)

                # normalize from PSUM on ACT: z = y*a + b
                zsb = lnp.tile([128, D], BF16, tag="zsb")
                for h in range(2):
                    nc.scalar.activation(
                        zsb[:, h * 512:(h + 1) * 512], ph[h][:],
                        mybir.ActivationFunctionType.Identity,
                        bias=ab[:, 1:2], scale=ab[:, 0:1])
                # gamma / beta
                zg = lnp.tile([128, D], BF16, tag="zg")
                nc.vector.tensor_tensor(zg[:], zsb[:], gammab[:],
                                        mybir.AluOpType.mult)
                zf = lnp.tile([128, D], F32, tag="zf")
                nc.vector.tensor_tensor(zf[:], zg[:], betab[:],
                                        mybir.AluOpType.add)
                nc.sync.dma_start(out[t * 128:(t + 1) * 128, :], zf[:])

    nc.compile()
    return nc


def _get_nc():
    if "nc" not in _cached:
        _cached["nc"] = _build()
    return _cached["nc"]


def kernel(x, mask, rank_weights, Ws, ln_gamma, ln_beta):
    nc = _get_nc()

    # host-side scalar path (16-element softmax)
    rw = rank_weights.astype(np.float64)
    e = np.exp(rw - rw.max())
    probs64 = e / e.sum()
    ranks = np.arange(MIN_RANK, MAX_RANK + 1, dtype=np.float64)
    expected_rank = np.float32((probs64 * ranks).sum())
    rank_entropy = np.float32(-(probs64 * np.log(probs64 + 1e-8)).sum())
    probs = probs64.astype(np.float32)

    x2 = np.ascontiguousarray(x.reshape(B * S, D))
    mask2 = mask.reshape(B * S)
    probs_col = np.ascontiguousarray(np.broadcast_to(probs[None, :], (128, R)))
    gamma_b = np.ascontiguousarray(
        np.broadcast_to(ln_gamma[None, :], (128, D))).astype(NP_BF16)
    beta_b = np.ascontiguousarray(
        np.broadcast_to(ln_beta[None, :], (128, D))).astype(NP_BF16)
    ident = np.eye(128, dtype=NP_BF16)

    in_maps = []
    for i in range(N_CORES):
        xs_i = x2[i * T:(i + 1) * T].astype(NP_BF16)
        ws_i = np.ascontiguousarray(
            Ws[:, i * OL:(i + 1) * OL, :]).astype(NP_BF16)
        m_i = np.ascontiguousarray(
            mask2[i * T:(i + 1) * T].reshape(TB, 128).T).astype(np.float32)
        in_maps.append({
            "xs": xs_i,
            "ws": ws_i,
            "probs": probs_col,
            "maskc": m_i,
            "gammab": gamma_b,
            "betab": beta_b,
            "ident": ident,
        })

    _cached["in_maps"] = in_maps
    res = bass_utils.run_bass_kernel_spmd(
        nc, in_maps, core_ids=list(range(N_CORES)))
    out = np.concatenate([res.results[i]["out"] for i in range(N_CORES)],
                         axis=0).reshape(B, S, D).astype(np.float32)
    return out, expected_rank, rank_entropy


# revision 14
# speedup vs baseline: 1.4187x; 1.4187x over previous
"""Distributed Trainium2 kernel for AdaptiveLowRank (softmax-weighted sum of
16 linear maps + LayerNorm), SPMD across 8 NeuronCores.

Strategy: data-parallel over tokens; the 16 weight matrices are sharded over
both r and output-dim so each core reads only 1/8 of Ws. The o-slice of
W_eff = sum_r softmax(rank_weights)_r * W_r is combined on the TensorEngine
(p_r*I diagonal matmuls accumulating in PSUM), AllGathered in two d-chunks
(overlapping the gather with the start of the main matmul), and the full
W_eff^T moving tiles plus the x^T stationary tiles are produced by xbar
DMA-transpose. LayerNorm stats run on DVE (bn_stats), the normalize runs on
ACT straight out of PSUM.
"""

import numpy as np
import ml_dtypes

import concourse.bass as bass
import concourse.mybir as mybir
import concourse.tile as tile
from concourse import bacc
from concourse import bass_utils

N_CORES = 8
B, S, D = 4, 2048, 1024
R = 16
T = (B * S) // N_CORES          # tokens per core
OL = D // N_CORES               # output rows owned per core
TB = T // 128                   # token blocks per core
DB = D // 128                   # contraction blocks
DH = D // 2                     # d-half size (AG chunk)
LN_EPS = 1e-5
MIN_RANK, MAX_RANK = 1, 16

BF16 = mybir.dt.bfloat16
F32 = mybir.dt.float32
NP_BF16 = ml_dtypes.bfloat16

_cached = {}


def _build(affine: bool):
    nc = bacc.Bacc("TRN2", target_bir_lowering=False, debug=False,
                   num_devices=N_CORES)

    xs = nc.dram_tensor("xs", [T, D], BF16, kind="ExternalInput")
    ws = nc.dram_tensor("ws", [R, OL, D], BF16, kind="ExternalInput")
    diag_in = nc.dram_tensor("diag", [128, R * 128], BF16,
                             kind="ExternalInput")
    maskc_in = nc.dram_tensor("maskc", [128, TB], F32, kind="ExternalInput")
    if affine:
        gamma_in = nc.dram_tensor("gammab", [128, D], BF16,
                                  kind="ExternalInput")
        beta_in = nc.dram_tensor("betab", [128, D], BF16,
                                 kind="ExternalInput")
    out = nc.dram_tensor("out", [T, D], F32, kind="ExternalOutput")

    with tile.TileContext(nc) as tc:
        with (
            tc.tile_pool(name="dram", bufs=1, space="DRAM") as dram,
            tc.tile_pool(name="consts", bufs=1) as consts,
            tc.tile_pool(name="wld", bufs=16) as wld,
            tc.tile_pool(name="accsb", bufs=2) as accsb,
            tc.tile_pool(name="xt", bufs=1) as xtp,
            tc.tile_pool(name="wmov", bufs=1) as wmov,
            tc.tile_pool(name="psum_w", bufs=2, space="PSUM") as psum_w,
            tc.tile_pool(name="psum_y", bufs=6, space="PSUM") as psum_y,
            tc.tile_pool(name="ln", bufs=3) as lnp,
            tc.tile_pool(name="stats", bufs=3) as stats,
        ):
            # ---- phase D first in program order: transpose x shard ----
            xt = []
            for k in range(DB):
                t_ = xtp.tile([128, T], BF16, tag=f"xt{k}")
                nc.sync.dma_start(t_[:], xs[:, k * 128:(k + 1) * 128],
                                  transpose=True)
                xt.append(t_)

            diag = consts.tile([128, R * 128], BF16, name="diag")
            nc.sync.dma_start(diag[:], diag_in[:])
            maskc = consts.tile([128, TB], F32, name="maskc")
            nc.sync.dma_start(maskc[:], maskc_in[:])
            if affine:
                gammab = consts.tile([128, D], BF16, name="gammab")
                nc.sync.dma_start(gammab[:], gamma_in[:])
                betab = consts.tile([128, D], BF16, name="betab")
                nc.sync.dma_start(betab[:], beta_in[:])

            # ---- phase A+B per d-half: combine on PE, AllGather ----
            cc_outs = []
            for h in range(2):
                wtiles = []
                for r in range(R):
                    w_r = wld.tile([OL, DH], BF16, tag="wld")
                    nc.sync.dma_start(
                        w_r[:], ws[r][:, h * DH:(h + 1) * DH])
                    wtiles.append(w_r)
                pw = psum_w.tile([128, DH], F32, tag="pw")
                for r in range(R):
                    nc.tensor.matmul(pw[:], diag[:, r * 128:(r + 1) * 128],
                                     wtiles[r][:],
                                     start=(r == 0), stop=(r == R - 1))
                acc_sb = accsb.tile([128, DH], BF16, tag="accsb")
                nc.scalar.copy(acc_sb[:], pw[:])
                cc_in = dram.tile([OL, DH], BF16, name=f"cc_in{h}")
                nc.sync.dma_start(cc_in[:], acc_sb[:])
                cc_out = dram.tile([N_CORES * OL, DH], BF16,
                                   name=f"cc_out{h}", addr_space="Shared")
                nc.gpsimd.collective_compute(
                    "AllGather",
                    mybir.AluOpType.bypass,
                    replica_groups=[list(range(N_CORES))],
                    ins=[cc_in[:].opt()],
                    outs=[cc_out[:].opt()],
                )
                cc_outs.append(cc_out)

            # ---- phase C: W_eff^T moving tiles via DMA-transpose ----
            wm = []
            for k in range(DB):
                h, kk = k // 4, k % 4
                t_ = wmov.tile([128, D], BF16, tag=f"wm{k}")
                nc.sync.dma_start(
                    t_[:], cc_outs[h][:, kk * 128:(kk + 1) * 128],
                    transpose=True)
                wm.append(t_)

            # ---- phase E+F: matmuls + LayerNorm per token block ----
            for t in range(TB):
                ph0 = psum_y.tile([128, 512], F32, tag="py")
                ph1 = psum_y.tile([128, 512], F32, tag="py")
                ph = [ph0, ph1]
                for k in range(DB):
                    lhsT = xt[k][:, t * 128:(t + 1) * 128]
                    for h in range(2):
                        nc.tensor.matmul(ph[h][:], lhsT,
                                         wm[k][:, h * 512:(h + 1) * 512],
                                         start=(k == 0), stop=(k == DB - 1))

                # LN stats on DVE: bn_stats per half -> bn_aggr
                bs = stats.tile([128, 12], F32, tag="bs")
                for h in range(2):
                    nc.vector.bn_stats(bs[:, h * 6:(h + 1) * 6], ph[h][:])
                mv = stats.tile([128, 8], F32, tag="mv")
                nc.vector.bn_aggr(mv[:, 0:2], bs[:])
                # a_pre = m^2*var + eps; rs = 1/sqrt(a_pre)
                m = maskc[:, t:t + 1]
                nc.vector.tensor_scalar(
                    mv[:, 2:3], m, m, None, mybir.AluOpType.mult)
                nc.vector.tensor_scalar(
                    mv[:, 3:4], mv[:, 1:2], mv[:, 2:3], LN_EPS,
                    mybir.AluOpType.mult, mybir.AluOpType.add)
                nc.scalar.sqrt(mv[:, 4:5], mv[:, 3:4])
                nc.vector.reciprocal(mv[:, 5:6], mv[:, 4:5])
                # a = m*rs ; b = -mean*a
                nc.vector.tensor_scalar(
                    mv[:, 6:7], mv[:, 5:6], m, None, mybir.AluOpType.mult)
                nc.vector.tensor_scalar(
                    mv[:, 7:8], mv[:, 6:7], mv[:, 0:1], -1.0,
                    mybir.AluOpType.mult, mybir.AluOpType.mult)

                # normalize from PSUM on ACT: z = y*a + b
                if affine:
                    zsb = lnp.tile([128, D], BF16, tag="zsb")
                    for h in range(2):
                        nc.scalar.activation(
                            zsb[:, h * 512:(h + 1) * 512], ph[h][:],
                            mybir.ActivationFunctionType.Identity,
                            bias=mv[:, 7:8], scale=mv[:, 6:7])
                    zg = lnp.tile([128, D], BF16, tag="zg")
                    nc.vector.tensor_tensor(zg[:], zsb[:], gammab[:],
                                            mybir.AluOpType.mult)
                    zf = lnp.tile([128, D], F32, tag="zf")
                    nc.vector.tensor_tensor(zf[:], zg[:], betab[:],
                                            mybir.AluOpType.add)
                else:
                    zf = lnp.tile([128, D], F32, tag="zf")
                    for h in range(2):
                        nc.scalar.activation(
                            zf[:, h * 512:(h + 1) * 512], ph[h][:],
                            mybir.ActivationFunctionType.Identity,
                            bias=mv[:, 7:8], scale=mv[:, 6:7])
                nc.sync.dma_start(out[t * 128:(t + 1) * 128, :], zf[:])

    nc.compile()
    return nc


def _get_nc(affine: bool):
    key = f"nc_{affine}"
    if key not in _cached:
        _cached[key] = _build(affine)
    return _cached[key]


def _host_prep(x, mask, rank_weights, Ws, ln_gamma, ln_beta):
    rw = rank_weights.astype(np.float64)
    e = np.exp(rw - rw.max())
    probs64 = e / e.sum()
    ranks = np.arange(MIN_RANK, MAX_RANK + 1, dtype=np.float64)
    expected_rank = np.float32((probs64 * ranks).sum())
    rank_entropy = np.float32(-(probs64 * np.log(probs64 + 1e-8)).sum())
    probs = probs64.astype(np.float32)

    affine = not (np.all(ln_gamma == 1.0) and np.all(ln_beta == 0.0))

    x2 = np.ascontiguousarray(x.reshape(B * S, D))
    mask2 = mask.reshape(B * S)
    diag = np.zeros((128, R * 128), dtype=np.float32)
    for r in range(R):
        diag[np.arange(128), r * 128 + np.arange(128)] = probs[r]
    diag = diag.astype(NP_BF16)
    if affine:
        gamma_b = np.ascontiguousarray(
            np.broadcast_to(ln_gamma[None, :], (128, D))).astype(NP_BF16)
        beta_b = np.ascontiguousarray(
            np.broadcast_to(ln_beta[None, :], (128, D))).astype(NP_BF16)

    in_maps = []
    for i in range(N_CORES):
        m = {
            "xs": x2[i * T:(i + 1) * T].astype(NP_BF16),
            "ws": np.ascontiguousarray(
                Ws[:, i * OL:(i + 1) * OL, :]).astype(NP_BF16),
            "diag": diag,
            "maskc": np.ascontiguousarray(
                mask2[i * T:(i + 1) * T].reshape(TB, 128).T
            ).astype(np.float32),
        }
        if affine:
            m["gammab"] = gamma_b
            m["betab"] = beta_b
        in_maps.append(m)
    return in_maps, affine, expected_rank, rank_entropy


def kernel(x, mask, rank_weights, Ws, ln_gamma, ln_beta):
    in_maps, affine, expected_rank, rank_entropy = _host_prep(
        x, mask, rank_weights, Ws, ln_gamma, ln_beta)
    nc = _get_nc(affine)
    _cached["in_maps"] = in_maps
    _cached["affine"] = affine
    res = bass_utils.run_bass_kernel_spmd(
        nc, in_maps, core_ids=list(range(N_CORES)))
    out = np.concatenate([res.results[i]["out"] for i in range(N_CORES)],
                         axis=0).reshape(B, S, D).astype(np.float32)
    return out, expected_rank, rank_entropy
